# revision 1
# baseline (speedup 1.0000x reference)
"""ECG spiking encoder (conv-tokenizer + 2x {linear, parametric-LIF} + time-mean)
as a Bass kernel on 8 TRN2 NeuronCores, pure data parallel over batch.

Math (per core, batch shard of 64):
  patches   = im2col(x)                      # stride==kernel -> pure relayout
  h1        = patches @ Wc.T + bc            # conv fused with fc1 (host weight fold)
  u1        = sig1*h1 + sig1*bc              # folded into GEMM weights + epilogue bias
  LIF1      : v <- v + (h1 - v)*sig1 ; s = H(v-1) ; v <- v - s
  h2/u2     = fc2(s1) ...
  LIF2      ; out = mean_t(s2)

Device mapping:
  GEMM1: bf16 hi/lo 3-pass (exact to ~2^-16) over k=640 (5 chunks of 128, zero pad),
         x packed per row-tile on host so each tile is two large DMAs
  GEMM2: bf16 hi/lo 2-pass (spikes are exact in bf16)
  LIF   : one fused custom DVE op per step, both layers merged in one [128,2,64] tile
          state v'_t = (v'_{t-1} - (v'_{t-1} > 1)) * a + u_t   (v'-form, u pre-scaled;
          layer 2 lags layer 1 by LAG steps so GEMM2/epilogue latency is hidden)
  spikes: bulk  s = (v' > 1) extraction (consistent strict >); mean via tensor_reduce.
"""
import numpy as np
import ml_dtypes
from contextlib import ExitStack

import concourse.bass as bass
import concourse.tile as tile
from concourse import bacc, mybir
from concourse.bass_utils import run_bass_kernel_spmd

F32 = mybir.dt.float32
F32R = mybir.dt.float32r
BF16 = mybir.dt.bfloat16

# 13 row tiles of 8 t-steps (last: 4); x packed per tile, chunk-major
TILES = [(j * 512, 512 if j < 12 else 256) for j in range(13)]

# ---- problem constants (hardcoded per contract) ----
B, C, L = 512, 12, 5000
E, H1, H2, P = 128, 128, 128, 50
T = 100
STRIDE = 50
V_TH = 1.0
NCORES = 8
BS = B // NCORES          # 64 batch per core
K = C * P                 # 600 contraction
KPAD = 640                # 5 chunks of 128
NCH = KPAD // 128         # 5
NT = 13                   # row tiles: 12x512 + 1x256
ROWS = T * BS             # 6400
LAG = 16                  # layer-2 lag in steps (multiple of 8)
NBLK = T // 8             # 12.5 -> handled as 13 blocks (last half)
MSTEPS = T + LAG          # 108 merged scan steps


def _register_lif_op():
    """Fused LIF step as a custom DVE op, via the documented extension point
    (concourse dve_ops registry). Idempotent across calls."""
    import concourse.dve_ops as dom
    from concourse.dve_spec import Spec, Src0, Src1, C0, C1, lower, _has_src1
    from concourse.dve_uop import DveOpSpec

    name = "LIF_EMA_RESET_STEP"
    for op in dom.OPS:
        if op.name == name:
            return op

    body = (Src0 - (Src0 > C1)) * C0 + Src1

    def ref(in0, in1, s0, s1, imm2):
        return (((in0 - (in0 > s1)) * s0) + in1).astype(np.float32)

    spec = Spec(body=body, reference=ref)
    row = dom._CUSTOM_DVE_ROW_BASE + len(dom.OPS)
    assert row < 0x20
    shas = {}
    for ver in ("v3", "v4"):
        uops = lower(spec, ver=ver)
        shas[ver] = DveOpSpec(name=name, opcode=row, uops=uops,
                              rd1_en=_has_src1(spec)).sha(ver)
    op = dom.DveOp(name, spec, subdim=False, uops_sha=shas)
    dom.OPS.append(op)
    dom._SUB_OPCODE_FOR_NAME[name] = row
    dom.CUSTOM_DVE_SPECS[name] = spec
    return op


def _build_program(a1: float, a2: float):
    lif_op = _register_lif_op()
    nc = bacc.Bacc("TRN2", target_bir_lowering=False, debug=False,
                   num_devices=NCORES)

    xhl_d = nc.dram_tensor("xhl", [128, 10 * ROWS], BF16, kind="ExternalInput").ap()
    w1_d = nc.dram_tensor("w1", [128, 12 * H1], BF16, kind="ExternalInput").ap()
    b1_d = nc.dram_tensor("b1", [128, 2], F32, kind="ExternalInput").ap()
    out_d = nc.dram_tensor("out", [128, BS], F32, kind="ExternalOutput").ap()

    # GEMM1 3-pass MM schedule: hi*Wh (5), hi*Wl (5), lo*Wh (5) --
    # first ten matmuls read only the hi chunks (first DMA half of each tile)
    W_IDX = [0, 1, 2, 3, 4] + [5, 6, 7, 8, 9] + [0, 1, 2, 3, 4]
    X_IDX = [0, 1, 2, 3, 4] + [0, 1, 2, 3, 4] + [5, 6, 7, 8, 9]

    # staircase column groups (module-level GROUPS): small first so the scan
    # starts early, small last so the post-stream tail is short
    HALF = MSTEPS * 64          # vball column offset of the layer-2 half

    with tile.TileContext(nc) as tc, ExitStack() as ctx:
        wpool = ctx.enter_context(tc.tile_pool(name="wpool", bufs=1))
        xpool = ctx.enter_context(tc.tile_pool(name="xpool", bufs=8))
        upool = ctx.enter_context(tc.tile_pool(name="upool", bufs=8))
        spool = ctx.enter_context(tc.tile_pool(name="spool", bufs=3))
        vpool = ctx.enter_context(tc.tile_pool(name="vpool", bufs=1))
        ps1pool = ctx.enter_context(tc.tile_pool(name="ps1", bufs=4, space="PSUM"))
        ps2pool = ctx.enter_context(tc.tile_pool(name="ps2", bufs=2, space="PSUM"))
        mpool = ctx.enter_context(tc.tile_pool(name="mpool", bufs=1))

        # weights
        wall = wpool.tile([128, 12 * H1], BF16)
        nc.gpsimd.dma_start(wall[:], w1_d[:])
        wt = wall[:, 0:10 * H1]
        w2t = wall[:, 10 * H1:12 * H1]
        ball = wpool.tile([128, 2], F32)
        nc.gpsimd.dma_start(ball[:], b1_d[:])
        b1t = ball[:, 0:1]
        b2t = ball[:, 1:2]

        # merged v' trajectory, split halves: L1 at cols [0, HALF), L2 at [HALF, 2*HALF)
        vball = vpool.tile([128, 2 * HALF], F32)
        vb2 = vball[:].rearrange("p (h q) -> p h q", h=2)
        zinit = wpool.tile([128, 128], F32)
        nc.vector.memset(zinit[:], 0.0)

        # u blocks: [128, 1024] = u1 (8 steps x 64) | u2 (8 steps x 64)
        ublks = [None] * (14 + LAG // 8)

        def ublk_for(k):
            if ublks[k] is None:
                t_ = upool.tile([128, 1024], F32, tag="ublk", name=f"ublk{k}")
                ublks[k] = t_
            return ublks[k]

        m_done = 0
        merged = float(a1) == float(a2)

        def emit_scan_through(m_end):
            nonlocal m_done
            while m_done < m_end:
                m = m_done
                ub = ublks[m // 8]
                ub2 = ub[:].rearrange("p (h q) -> p h q", h=2)
                s = m % 8
                if merged:
                    src = (zinit[:].rearrange("p (h q) -> p h q", h=2) if m == 0
                           else vb2[:, :, (m - 1) * 64:m * 64])
                    nc.vector._custom_dve(
                        lif_op, out=vb2[:, :, m * 64:(m + 1) * 64], in0=src,
                        in1=ub2[:, :, s * 64:(s + 1) * 64], s0=a1, s1=V_TH)
                else:
                    for h, a_ in ((0, a1), (1, a2)):
                        src = (zinit[:, 0:64] if m == 0
                               else vball[:, h * HALF + (m - 1) * 64:h * HALF + m * 64])
                        nc.vector._custom_dve(
                            lif_op,
                            out=vball[:, h * HALF + m * 64:h * HALF + (m + 1) * 64],
                            in0=src,
                            in1=ub[:, h * 512 + s * 64:h * 512 + (s + 1) * 64],
                            s0=a_, s1=V_TH)
                m_done += 1

        tail1_done = False
        tail2_done = False

        flat_offs = []
        off = 0
        for (c0, ncols) in TILES:
            flat_offs.append(off)
            off += 10 * ncols

        xgs = {}
        pss = {}

        def emit_front(j):
            # DMA + GEMM1 matmuls for tile j (paired tiles share weight loads)
            (c0_, nc_) = TILES[j]
            fo = flat_offs[j]
            half = 5 * nc_
            xg = xpool.tile([128, 10 * 512], BF16, tag="xg", name=f"xg{j}")
            nc.sync.dma_start(xg[:, 0:half], xhl_d[:, fo:fo + half])
            nc.sync.dma_start(xg[:, half:2 * half], xhl_d[:, fo + half:fo + 2 * half])
            xgs[j] = xg
            pss[j] = ps1pool.tile([128, nc_], F32, tag="ps1t", name=f"ps{j}")

        def emit_mms(js):
            for i in range(15):
                for j_ in js:
                    (c0_, nc_) = TILES[j_]
                    nc.tensor.matmul(
                        pss[j_][:], wt[:, bass.ts(W_IDX[i], H1)],
                        xgs[j_][:, X_IDX[i] * nc_: X_IDX[i] * nc_ + nc_],
                        start=(i == 0), stop=(i == 14))

        # tile 0 solo (fast start), then pairs
        SCHED = [[0]] + [[j, j + 1] if j + 1 < NT else [j] for j in range(1, NT, 2)]
        for js in SCHED:
            for j_ in js:
                emit_front(j_)
            emit_mms(js)
            for j in js:
                (col0, ncols) = TILES[j]
                nsteps = ncols // 64
                ps = pss[j]
                # epilogue 1 -> u1 half of block j
                ub = ublk_for(j)
                if j < LAG // 8:
                    nc.vector.memset(ub[:, 512:1024], 0.0)  # u2 of first blocks = 0
                nc.scalar.activation(
                    ub[:].rearrange("p (s c) -> p s c", c=64)[:, :nsteps],
                    ps[:].rearrange("p (s c) -> p s c", c=64),
                    mybir.ActivationFunctionType.Identity, bias=b1t[:, 0:1])
                if j == 12:
                    nc.vector.memset(ub[:, nsteps * 64:512], 0.0)

                emit_scan_through(min(8 * (j + 1), MSTEPS))

                # s1 extraction for block j (contiguous L1 half)
                sb = spool.tile([128, 512], BF16, tag="s1b", name=f"s1b{j}")
                nc.vector.tensor_scalar(
                    sb[:, :ncols], vball[:, 8 * j * 64:8 * j * 64 + ncols],
                    V_TH, None, mybir.AluOpType.is_gt, mybir.AluOpType.bypass)

                # GEMM2 (bf16 hi/lo) -> u2 for L2 steps 8j..; lands in u block j + LAG/8
                ps2 = ps2pool.tile([128, ncols], F32, tag="ps2t", name=f"ps2{j}")
                nc.tensor.matmul(ps2[:], w2t[:, 0:H2], sb[:, :ncols], start=True, stop=False)
                nc.tensor.matmul(ps2[:], w2t[:, H2:2 * H2], sb[:, :ncols], start=False, stop=True)
                ub_next = ublk_for(j + LAG // 8)
                if j + LAG // 8 >= 13:
                    nc.vector.memset(ub_next[:, 0:512], 0.0)  # u1 of tail blocks = 0
                nc.scalar.activation(
                    ub_next[:].rearrange("p (s c) -> p s c", c=64)[:, 8:8 + nsteps],
                    ps2[:].rearrange("p (s c) -> p s c", c=64),
                    mybir.ActivationFunctionType.Identity, bias=b2t[:, 0:1])
                if j == 12 and nsteps < 8:
                    nc.vector.memset(ub_next[:, 512 + nsteps * 64:1024], 0.0)

                # early partial layer-2 tails
                if m_done >= 52 + LAG + 2 and not tail1_done:
                    tail1_done = True
                    r0 = HALF + LAG * 64
                    nc.vector.tensor_scalar(
                        vball[:, r0:r0 + 3200], vball[:, r0:r0 + 3200],
                        V_TH, None, mybir.AluOpType.is_gt, mybir.AluOpType.bypass)
                    acc1 = mpool.tile([128, BS], F32, name="acc1")
                    nc.vector.tensor_reduce(
                        acc1[:],
                        vball[:, r0:r0 + 3200].rearrange("p (t b) -> p b t", b=64),
                        mybir.AxisListType.X, mybir.AluOpType.add)
                if m_done >= 104 and not tail2_done:
                    tail2_done = True
                    r0 = HALF + LAG * 64 + 3200
                    n2 = 38 * 64
                    nc.vector.tensor_scalar(
                        vball[:, r0:r0 + n2], vball[:, r0:r0 + n2],
                        V_TH, None, mybir.AluOpType.is_gt, mybir.AluOpType.bypass)
                    acc2 = mpool.tile([128, BS], F32, name="acc2")
                    nc.vector.tensor_reduce(
                        acc2[:],
                        vball[:, r0:r0 + n2].rearrange("p (t b) -> p b t", b=64),
                        mybir.AxisListType.X, mybir.AluOpType.add)
                    acc12 = mpool.tile([128, BS], F32, name="acc12")
                    nc.vector.scalar_tensor_tensor(
                        acc12[:], acc1[:], 1.0, acc2[:],
                        mybir.AluOpType.mult, mybir.AluOpType.add)

        emit_scan_through(MSTEPS)

        # last 12 t-steps of layer-2 spikes + mean
        r1 = HALF + LAG * 64 + 5632
        n3 = 12 * 64
        nc.vector.tensor_scalar(vball[:, r1:r1 + n3], vball[:, r1:r1 + n3],
                                V_TH, None, mybir.AluOpType.is_gt,
                                mybir.AluOpType.bypass)
        acc3 = mpool.tile([128, BS], F32, name="acc3")
        nc.vector.tensor_reduce(
            acc3[:], vball[:, r1:r1 + n3].rearrange("p (t b) -> p b t", b=64),
            mybir.AxisListType.X, mybir.AluOpType.add)
        acc = mpool.tile([128, BS], F32, name="accf")
        nc.vector.scalar_tensor_tensor(acc[:], acc12[:], 1.0, acc3[:],
                                       mybir.AluOpType.mult, mybir.AluOpType.add)
        nc.vector.tensor_scalar(acc[:], acc[:], float(np.float32(1.0 / T)), None,
                                mybir.AluOpType.mult, mybir.AluOpType.bypass)
        nc.sync.dma_start(out_d[:], acc[:])

    nc.compile()
    return nc


_PROG_CACHE = {}


def _get_program(a1, a2):
    key = (round(float(a1), 10), round(float(a2), 10))
    if key not in _PROG_CACHE:
        _PROG_CACHE[key] = _build_program(float(a1), float(a2))
    return _PROG_CACHE[key]


def prepare(x, conv_w, conv_b, fc1_w, fc1_b, fc2_w, fc2_b, w1, w2):
    """Host-side prep: weight folding, im2col relayout, per-core shards.
    Returns (a1, a2, in_maps)."""
    x = np.asarray(x, np.float32)
    conv_w = np.asarray(conv_w, np.float32)
    conv_b = np.asarray(conv_b, np.float32)
    fc1_w = np.asarray(fc1_w, np.float32)
    fc1_b = np.asarray(fc1_b, np.float32)
    fc2_w = np.asarray(fc2_w, np.float32)
    fc2_b = np.asarray(fc2_b, np.float32)

    sig1 = 1.0 / (1.0 + np.exp(-np.float64(w1)))
    sig2 = 1.0 / (1.0 + np.exp(-np.float64(w2)))
    a1 = np.float32(1.0 - sig1)
    a2 = np.float32(1.0 - sig2)
    sig1 = np.float32(sig1)
    sig2 = np.float32(sig2)

    # ---- weight folding (host, fp64 for exactness headroom) ----
    # u1 = sig1*(fc1_w @ (conv_w.x + conv_b) + fc1_b)
    Wc = sig1.astype(np.float64) * (fc1_w.astype(np.float64) @ conv_w.reshape(E, K).astype(np.float64))
    bc = sig1.astype(np.float64) * (fc1_w.astype(np.float64) @ conv_b.astype(np.float64) + fc1_b.astype(np.float64))
    Wc = Wc.astype(np.float32)                      # [H1, K]
    bc = bc.astype(np.float32)                      # [H1]
    Wcp = np.zeros((H1, KPAD), np.float32)
    Wcp[:, :K] = Wc
    # lhsT chunks [k, H1], bf16 hi/lo
    WcT = Wcp.T.copy()                              # [KPAD, H1]
    Wh = WcT.astype(ml_dtypes.bfloat16)
    Wl = (WcT - Wh.astype(np.float32)).astype(ml_dtypes.bfloat16)
    W2Tf = (sig2.astype(np.float64) * fc2_w.astype(np.float64)).T.astype(np.float32)  # [H1, H2] lhsT
    W2h = W2Tf.astype(ml_dtypes.bfloat16)
    W2l = (W2Tf - W2h.astype(np.float32)).astype(ml_dtypes.bfloat16)
    # packed [128, 12*H1]: 5 hi chunks | 5 lo chunks | w2 hi | w2 lo
    w1_arr = np.concatenate(
        [Wh.reshape(NCH, 128, H1).transpose(1, 0, 2).reshape(128, NCH * H1),
         Wl.reshape(NCH, 128, H1).transpose(1, 0, 2).reshape(128, NCH * H1),
         W2h, W2l], axis=1)
    b_arr = np.stack([bc, (sig2 * fc2_b).astype(np.float32)], axis=1)  # [128, 2]

    # ---- im2col + shard (pure relayout; stride == kernel width) ----
    # x [B, C, L] -> per-core [64, C, T, P] -> (c, p, t, b) -> [K, T*BS]
    in_maps = []
    for ci in range(NCORES):
        xs = x[ci * BS:(ci + 1) * BS].reshape(BS, C, T, P)
        xT = np.ascontiguousarray(xs.transpose(1, 3, 2, 0)).reshape(K, ROWS)
        xTp = np.zeros((KPAD, ROWS), np.float32)
        xTp[:K] = xT
        xh = xTp.astype(ml_dtypes.bfloat16)
        xl = (xTp - xh.astype(np.float32)).astype(ml_dtypes.bfloat16)
        chunks = np.concatenate([xh.reshape(NCH, 128, ROWS),
                                 xl.reshape(NCH, 128, ROWS)], axis=0)  # [10,128,ROWS]
        # pack tile-major / chunk-major
        parts = []
        for (c0, ncols) in TILES:
            parts.append(np.ascontiguousarray(
                chunks[:, :, c0:c0 + ncols].transpose(1, 0, 2).reshape(128, 10 * ncols)))
        xhl = np.concatenate(parts, axis=1)
        in_maps.append({"xhl": xhl, "w1": w1_arr, "b1": b_arr})

    return a1, a2, in_maps


def kernel(**inputs):
    a1, a2, in_maps = prepare(**inputs)
    nc = _get_program(a1, a2)
    res = run_bass_kernel_spmd(nc, in_maps, list(range(NCORES)))
    out = np.empty((B, H2), np.float32)
    for ci in range(NCORES):
        out[ci * BS:(ci + 1) * BS] = res.results[ci]["out"].T
    return out



# revision 11
# speedup vs baseline: 1.6236x; 1.6236x over previous
"""ECG spiking encoder (conv-tokenizer + 2x {linear, parametric-LIF} + time-mean)
as a Bass kernel on 8 TRN2 NeuronCores, pure data parallel over batch.

v3 design (per core, batch shard of 64):
  - x im2col'd, packed per row-tile, fp16 (half the HBM bytes of bf16 hi/lo);
    conv+fc1+sig1 folded into one [640,128] fp16 weight; u1 bias rides pad
    row 600 of x (constant 1.0), so GEMM1 emits u1 = sig1*h1 directly.
  - GEMM1: 5 matmuls per 512-col tile (K=640 in 5 chunks), fp32 PSUM.
  - The serial LIF chains are the critical path: one fused custom DVE op per
    merged step (layer-2 lagging layer-1 by LAG=16 steps, half-width ops in
    the head/tail). The DVE queue carries ONLY the scan. The op reads u
    DIRECTLY FROM PSUM (no epilogues anywhere): GEMM1 writes bank A and
    GEMM2 writes bank B of a shared [128,1024] psum tile; the merged op's
    in1 is one stepped [128,2,64] PSUM access pattern over both banks.
  - Spike extraction on the Scalar engine: sv = Sign(v - 1) in {-1,+1} bf16.
    GEMM2 consumes sv with folded W2/2; the implied constant input
    (sig2/2)*fc2@1 + b2 is accumulated into PSUM by a ones-matmul.
  - Layer-2 spike mean via identity-matmul PSUM accumulation of sv2 chunks;
    final fold + (x/200 + 0.5) correction on DVE at the very end.
"""
import numpy as np
import ml_dtypes
from contextlib import ExitStack

import concourse.bass as bass
import concourse.tile as tile
from concourse import bacc, mybir
from concourse.bass_utils import run_bass_kernel_spmd

F32 = mybir.dt.float32
F16 = mybir.dt.float16
BF16 = mybir.dt.bfloat16
ml_bf16 = ml_dtypes.bfloat16

# ---- problem constants (hardcoded per contract) ----
B, C, L = 512, 12, 5000
E, H1, H2, P = 128, 128, 128, 50
T = 100
STRIDE = 50
V_TH = 1.0
NCORES = 8
BS = B // NCORES          # 64 batch per core
K = C * P                 # 600 contraction
KPAD = 640                # 5 chunks of 128 (row 600 = u1-bias row)
NCH = KPAD // 128         # 5
LAG = 16                  # layer-2 lag in merged steps
MS = T + LAG              # 116 merged steps
TILES = [(8 * j, 512 if j < 12 else 256) for j in range(13)]  # (t0, ncols)
NT = len(TILES)
ROWS = T * BS             # 6400 tokens per core
# trajectory row layout (64-col units): L1 step m at row m (m in [0,T));
# row T is a zero pad ("L2 time -1"); L2 time tau at row T + 1 + tau.
L2OFF = T + 1 - LAG       # merged-step m -> L2 row m + L2OFF
NROWS = T + 1 + T         # 201 rows of 64 cols


def _register_lif_op():
    """Fused LIF step (is_ge soft reset) as a custom DVE op. Idempotent."""
    import concourse.dve_ops as dom
    from concourse.dve_spec import Spec, Src0, Src1, C0, C1, lower, _has_src1
    from concourse.dve_uop import DveOpSpec

    name = "LIF_EMA_RESET_STEP_GE"
    for op in dom.OPS:
        if op.name == name:
            return op

    body = (Src0 - (Src0 >= C1)) * C0 + Src1

    def ref(in0, in1, s0, s1, imm2):
        return (((in0 - (in0 >= s1)) * s0) + in1).astype(np.float32)

    spec = Spec(body=body, reference=ref)
    row = dom._CUSTOM_DVE_ROW_BASE + len(dom.OPS)
    assert row < 0x20
    shas = {}
    for ver in ("v3", "v4"):
        uops = lower(spec, ver=ver)
        shas[ver] = DveOpSpec(name=name, opcode=row, uops=uops,
                              rd1_en=_has_src1(spec)).sha(ver)
    op = dom.DveOp(name, spec, subdim=False, uops_sha=shas)
    dom.OPS.append(op)
    dom._SUB_OPCODE_FOR_NAME[name] = row
    dom.CUSTOM_DVE_SPECS[name] = spec
    return op


def _build_program(a1: float, a2: float):
    lif_op = _register_lif_op()
    nc = bacc.Bacc("TRN2", target_bir_lowering=False, debug=False,
                   num_devices=NCORES)

    xt_d = nc.dram_tensor("xt", [128, NCH * ROWS], F16, kind="ExternalInput").ap()
    # weights: 5 fp16 w1 chunks | bf16 w2 | bf16 cw | bf16 ident (bit-packed)
    wt_d = nc.dram_tensor("wt", [128, (NCH + 3) * 128], F16, kind="ExternalInput").ap()
    out_d = nc.dram_tensor("out", [128, BS], F32, kind="ExternalOutput").ap()

    merged = float(a1) == float(a2)

    with tile.TileContext(nc) as tc, ExitStack() as ctx:
        wpool = ctx.enter_context(tc.tile_pool(name="wpool", bufs=1))
        xpool = ctx.enter_context(tc.tile_pool(name="xpool", bufs=4))
        spool = ctx.enter_context(tc.tile_pool(name="spool", bufs=3))
        s2pool = ctx.enter_context(tc.tile_pool(name="s2pool", bufs=2))
        tpool = ctx.enter_context(tc.tile_pool(name="tpool", bufs=1))
        mpool = ctx.enter_context(tc.tile_pool(name="mpool", bufs=1))
        pspool = ctx.enter_context(tc.tile_pool(name="ps", bufs=3, space="PSUM"))
        psapool = ctx.enter_context(tc.tile_pool(name="psa", bufs=1, space="PSUM"))

        wall = wpool.tile([128, (NCH + 3) * 128], F16)
        nc.sync.dma_start(wall[:], wt_d[:])
        w1t = wall[:, 0:NCH * 128]
        w2t = wall[:, NCH * 128:(NCH + 1) * 128].bitcast(BF16)
        cwt = wall[:, (NCH + 1) * 128:(NCH + 2) * 128].bitcast(BF16)
        ident = wall[:, (NCH + 2) * 128:(NCH + 3) * 128].bitcast(BF16)

        ones = wpool.tile([128, 512], BF16)
        nc.vector.memset(ones[:], 1.0)
        zinit = wpool.tile([128, 64], F32)
        nc.vector.memset(zinit[:], 0.0)
        nbias = wpool.tile([128, 1], F32)
        nc.vector.memset(nbias[:], -float(V_TH))

        # v trajectory (SBUF): written by the scan, read by Sign extractions
        traj = tpool.tile([128, NROWS * 64], F32)
        tv = traj[:].rearrange("p (n c) -> p n c", c=64)
        nc.vector.memset(traj[:, T * 64:(T + 1) * 64], 0.0)  # L2 "time -1"

        acc_ps = psapool.tile([128, 512], F32)

        # psum tiles: bank A (cols 0:512) = u1 of tile k; bank B (cols
        # 512:1024) = u2 for merged slots 8k..8k+8 (written by GEMM2(k-2)).
        # k = 13, 14 are virtual tail tiles (bank B only).
        pss = {}
        xgs = {}

        def emit_front(k):
            pss[k] = pspool.tile([128, 1024], F32, tag="pst", name=f"ps{k}")
            if k >= NT:
                return
            (t0, ncols) = TILES[k]
            xg = xpool.tile([128, NCH * 512], F16, tag="xg", name=f"xg{k}")
            nc.sync.dma_start(xg[:, 0:NCH * ncols],
                              xt_d[:, NCH * 64 * t0:NCH * 64 * t0 + NCH * ncols])
            xgs[k] = xg
            for c in range(NCH):
                nc.tensor.matmul(
                    pss[k][:, 0:ncols], w1t[:, bass.ts(c, 128)],
                    xg[:, c * ncols:(c + 1) * ncols],
                    start=(c == 0), stop=(c == NCH - 1))

        m_done = 0

        def emit_scan_through(m_end):
            nonlocal m_done
            while m_done < m_end:
                m = m_done
                j, s = m // 8, m % 8
                psv = pss[j][:].rearrange("p (n c) -> p n c", c=64)
                if m < LAG:                      # L1 only
                    pairs = [(m, psv[:, s, :], a1)]
                elif m < T:                      # merged (or split if a1!=a2)
                    if merged:
                        pairs = [(slice(m, m + L2OFF + 1, L2OFF),
                                  psv[:, s:s + 9:8, :], a1)]
                    else:
                        pairs = [(m, psv[:, s, :], a1),
                                 (m + L2OFF, psv[:, 8 + s, :], a2)]
                else:                            # L2 only (tail)
                    pairs = [(m + L2OFF, psv[:, 8 + s, :], a2)]
                for (row, u_ap, a_) in pairs:
                    if isinstance(row, slice):
                        o = tv[:, row, :]
                        i0 = tv[:, slice(m - 1, m - 1 + L2OFF + 1, L2OFF), :]
                    else:
                        o = traj[:, row * 64:(row + 1) * 64]
                        i0 = (zinit[:, 0:64] if m == 0
                              else traj[:, (row - 1) * 64:row * 64])
                    nc.vector._custom_dve(lif_op, out=o, in0=i0, in1=u_ap,
                                          s0=a_, s1=V_TH)
                m_done += 1

        l2_chunk = 0

        def emit_l2_ready():
            nonlocal l2_chunk
            while l2_chunk < NT:
                ct0, cn = TILES[l2_chunk]
                ns = cn // 64
                if ct0 + LAG + ns > m_done:
                    break
                sv2 = s2pool.tile([128, 512], BF16, tag="s2b", name=f"s2b{l2_chunk}")
                nc.scalar.activation(
                    sv2[:, 0:cn],
                    traj[:, (T + 1 + ct0) * 64:(T + 1 + ct0) * 64 + cn],
                    mybir.ActivationFunctionType.Sign, bias=nbias[:, 0:1])
                nc.tensor.matmul(acc_ps[:, 0:cn], ident[:], sv2[:, 0:cn],
                                 start=(l2_chunk == 0), stop=(l2_chunk == NT - 1))
                l2_chunk += 1

        for j in range(NT):
            (t0, ncols) = TILES[j]
            nsteps = ncols // 64
            if j == 0:
                for k in (0, 1, 2):
                    emit_front(k)
            else:
                emit_front(j + 2)

            emit_scan_through(t0 + nsteps)

            # L1 spike extraction: sv1 = sign(v1 - 1) in {-1,+1}, bf16
            sv1 = spool.tile([128, 512], BF16, tag="s1b", name=f"s1b{j}")
            nc.scalar.activation(
                sv1[:, 0:ncols], traj[:, t0 * 64:t0 * 64 + ncols],
                mybir.ActivationFunctionType.Sign, bias=nbias[:, 0:1])

            # GEMM2 on sv1 (W2/2) + ones-matmul constant -> bank B of ps[j+2]
            tgt = pss[j + 2][:, 512:512 + ncols]
            nc.tensor.matmul(tgt, w2t[:], sv1[:, 0:ncols], start=True, stop=False)
            nc.tensor.matmul(tgt, cwt[:], ones[:, 0:ncols], start=False, stop=True)

            emit_l2_ready()

        emit_scan_through(MS)
        emit_l2_ready()

        # fold sv2 accumulator over the 8 step-slots; mean = x/200 + 0.5
        macc = mpool.tile([128, BS], F32)
        nc.vector.tensor_reduce(
            macc[:], acc_ps[:].rearrange("p (s c) -> p c s", c=64),
            mybir.AxisListType.X, mybir.AluOpType.add)
        nc.vector.tensor_scalar(macc[:], macc[:], float(np.float32(1.0 / (2 * T))),
                                0.5, mybir.AluOpType.mult, mybir.AluOpType.add)
        nc.sync.dma_start(out_d[:], macc[:])

    nc.compile()
    return nc


_PROG_CACHE = {}


def _get_program(a1, a2):
    key = (round(float(a1), 10), round(float(a2), 10))
    if key not in _PROG_CACHE:
        _PROG_CACHE[key] = _build_program(float(a1), float(a2))
    return _PROG_CACHE[key]


def prepare(x, conv_w, conv_b, fc1_w, fc1_b, fc2_w, fc2_b, w1, w2):
    """Host-side prep: weight folding, im2col relayout, fp16 cast, shards."""
    x = np.asarray(x, np.float32)
    conv_w = np.asarray(conv_w, np.float32)
    conv_b = np.asarray(conv_b, np.float32)
    fc1_w = np.asarray(fc1_w, np.float32)
    fc1_b = np.asarray(fc1_b, np.float32)
    fc2_w = np.asarray(fc2_w, np.float32)
    fc2_b = np.asarray(fc2_b, np.float32)

    sig1 = 1.0 / (1.0 + np.exp(-np.float64(w1)))
    sig2 = 1.0 / (1.0 + np.exp(-np.float64(w2)))
    a1 = np.float32(1.0 - sig1)
    a2 = np.float32(1.0 - sig2)

    # fold conv+fc1 (+sig1); u1 bias rides pad row 600
    Wc = sig1 * (fc1_w.astype(np.float64) @ conv_w.reshape(E, K).astype(np.float64))
    bc = sig1 * (fc1_w.astype(np.float64) @ conv_b.astype(np.float64)
                 + fc1_b.astype(np.float64))
    WcT = np.zeros((KPAD, H1), np.float32)
    WcT[:K] = Wc.astype(np.float32).T
    WcT[K] = bc.astype(np.float32)
    w1_16 = WcT.astype(np.float16)                                   # [640, 128]

    # GEMM2 consumes sv1 in {-1,+1}: lhsT = (sig2/2 * fc2).T; the constant
    # input c' = sig2/2 * fc2 @ 1 + b2 rides a ones-matmul with cw = c'/128
    W2h = (0.5 * sig2 * fc2_w.astype(np.float64)).T
    w2_bf = W2h.astype(np.float32).astype(ml_bf16)
    cprime = (0.5 * sig2 * fc2_w.astype(np.float64).sum(axis=1)
              + sig2 * fc2_b.astype(np.float64))
    cw_bf = np.ascontiguousarray(
        np.broadcast_to((cprime / 128.0).astype(np.float32), (128, H2))
    ).astype(ml_bf16)
    id_bf = np.eye(128, dtype=np.float32).astype(ml_bf16)

    def bf_as_f16(a):
        return np.ascontiguousarray(a).view(np.uint16).view(np.float16)

    wt_arr = np.concatenate(
        [w1_16.reshape(NCH, 128, H1).transpose(1, 0, 2).reshape(128, NCH * H1),
         bf_as_f16(w2_bf), bf_as_f16(cw_bf), bf_as_f16(id_bf)], axis=1)

    # im2col + shard: x [B, C, L] -> per-core [KPAD, T*BS] fp16, tile-packed
    in_maps = []
    for ci in range(NCORES):
        xs = x[ci * BS:(ci + 1) * BS].reshape(BS, C, T, P)
        xT = np.ascontiguousarray(xs.transpose(1, 3, 2, 0)).reshape(K, ROWS)
        xTp = np.zeros((KPAD, ROWS), np.float16)
        xTp[:K] = xT.astype(np.float16)
        xTp[K] = np.float16(1.0)                                     # bias row
        chunks = xTp.reshape(NCH, 128, ROWS)
        parts = []
        for (t0, ncols) in TILES:
            c0 = t0 * 64
            parts.append(np.ascontiguousarray(
                chunks[:, :, c0:c0 + ncols].transpose(1, 0, 2)
            ).reshape(128, NCH * ncols))
        xt = np.concatenate(parts, axis=1)
        in_maps.append({"xt": xt, "wt": wt_arr})

    return a1, a2, in_maps


def kernel(**inputs):
    a1, a2, in_maps = prepare(**inputs)
    nc = _get_program(a1, a2)
    res = run_bass_kernel_spmd(nc, in_maps, list(range(NCORES)))
    out = np.empty((B, H2), np.float32)
    for ci in range(NCORES):
        out[ci * BS:(ci + 1) * BS] = res.results[ci]["out"].T
    return out
